# revision 4
# baseline (speedup 1.0000x reference)
import sys

if '/opt/trn_rl_repo' not in sys.path:
    sys.path.insert(0, '/opt/trn_rl_repo')

import numpy as np

# Model dims (hardcoded from the problem spec)
B, C, N = 4, 512, 2048
NH, D = 8, 64          # heads, head dim
HID = 1024             # mlp hidden
NLOC = N // 2          # sequence half per core
CG = C // 128          # channel groups of 128
MT = N // 128          # m-tiles of 128 over full sequence
BN_EPS = 1e-5

# fp8 scaling: weights x16, attnout x64 (ones col = 16/64)
WS = 16.0              # weight scale for fp8
AS = 64.0              # attnout scale
ONESV = WS / AS        # ones-column value -> attnout comes out x AS
SC_EXP = 1.0 / (WS * WS * (D ** 0.5))   # undo q,k weight scales + sqrt(D)

_CACHE = {}


def _build_nc(repeat=1):
    import concourse.bacc as bacc
    import concourse.bass as bass
    import concourse.tile as tile
    import concourse.mybir as mybir
    from contextlib import ExitStack

    F32R, F32 = mybir.dt.float32r, mybir.dt.float32
    BF16 = mybir.dt.bfloat16
    FP8 = mybir.dt.float8e4
    AF = mybir.ActivationFunctionType
    ALU = mybir.AluOpType
    DR = mybir.MatmulPerfMode.DoubleRow

    nc = bacc.Bacc("TRN2")

    x8_d = nc.dram_tensor("x8", [C, N], FP8, kind="ExternalInput")
    xbn_d = nc.dram_tensor("xbn", [C, NLOC], F32R, kind="ExternalInput")
    wq_d = nc.dram_tensor("wqT", [C, C], FP8, kind="ExternalInput")
    wk_d = nc.dram_tensor("wkT", [C, C], FP8, kind="ExternalInput")
    wv_d = nc.dram_tensor("wvT", [C, C], FP8, kind="ExternalInput")
    wp_d = nc.dram_tensor("wpT", [C, C], FP8, kind="ExternalInput")
    w1_d = nc.dram_tensor("w1T", [C, HID], FP8, kind="ExternalInput")
    w2_d = nc.dram_tensor("w2T", [HID, C], FP8, kind="ExternalInput")
    bns_d = nc.dram_tensor("bns", [C, 1], F32, kind="ExternalInput")    # gamma*rsqrt(var+eps)
    bnb_d = nc.dram_tensor("bnb", [C, 1], F32, kind="ExternalInput")    # beta - mean*bns
    s1_d = nc.dram_tensor("s1", [C, 1], F32, kind="ExternalInput")      # bns / (WS*AS)
    s2_d = nc.dram_tensor("s2", [C, 1], F32, kind="ExternalInput")      # bns / (WS*WS)
    ones_d = nc.dram_tensor("ones", [128, MT * NH], BF16, kind="ExternalInput")
    y_d = nc.dram_tensor("y", [C, NLOC], F32, kind="ExternalOutput")

    def emit_body(tc, pers_tiles):
        xbn, bns_sb, bnb_sb, s1_sb, s2_sb, attnout8, z1 = pers_tiles

        with tc.tile_pool(name="attn_data", bufs=1) as ad:
            x8 = ad.tile([128, CG, N], FP8)
            for g in range(CG):
                nc.gpsimd.dma_start(out=x8[:, g, :], in_=x8_d[g * 128:(g + 1) * 128, :])
            k_sb = ad.tile([128, CG, N], BF16)
            q_sb = ad.tile([128, CG, NLOC], BF16)
            vT = ad.tile([128, MT, NH * 65], BF16)
            # ones columns of vT (col 64 of each 65-wide head block)
            vT_ones = vT.rearrange("p m (h e) -> p (m h) e", e=65)[:, :, 64:65]
            nc.sync.dma_start(out=vT_ones,
                              in_=ones_d[:, :].rearrange("p (a b) -> p a b", b=1))

            # ---------------- Phase 1: qkv (fp8 DoubleRow) ----------------
            with tc.tile_pool(name="qkvw", bufs=1) as qw, \
                 tc.tile_pool(name="ps1", bufs=6, space="PSUM") as ps1:
                wq_sb = qw.tile([128, CG, C], FP8)
                wk_sb = qw.tile([128, CG, C], FP8)
                wv_sb = qw.tile([128, CG, C], FP8)
                for c in range(CG):
                    nc.gpsimd.dma_start(out=wq_sb[:, c, :], in_=wq_d[c * 128:(c + 1) * 128, :])
                    nc.gpsimd.dma_start(out=wk_sb[:, c, :], in_=wk_d[c * 128:(c + 1) * 128, :])
                    nc.gpsimd.dma_start(out=wv_sb[:, c, :], in_=wv_d[c * 128:(c + 1) * 128, :])

                # k over the full sequence
                for g in range(CG):
                    for mc in range(N // 512):
                        ps = ps1.tile([128, 512], F32, tag="ps1")
                        for cc in range(CG // 2):
                            nc.tensor.matmul(
                                ps,
                                wk_sb[:, 2 * cc:2 * cc + 2, g * 128:(g + 1) * 128],
                                x8[:, 2 * cc:2 * cc + 2, mc * 512:(mc + 1) * 512],
                                start=(cc == 0), stop=(cc == CG // 2 - 1),
                                perf_mode=DR)
                        nc.vector.tensor_copy(out=k_sb[:, g, mc * 512:(mc + 1) * 512], in_=ps)
                # q over local half
                for g in range(CG):
                    for qc in range(NLOC // 512):
                        ps = ps1.tile([128, 512], F32, tag="ps1")
                        for cc in range(CG // 2):
                            nc.tensor.matmul(
                                ps,
                                wq_sb[:, 2 * cc:2 * cc + 2, g * 128:(g + 1) * 128],
                                x8[:, 2 * cc:2 * cc + 2, qc * 512:(qc + 1) * 512],
                                start=(cc == 0), stop=(cc == CG // 2 - 1),
                                perf_mode=DR)
                        nc.vector.tensor_copy(out=q_sb[:, g, qc * 512:(qc + 1) * 512], in_=ps)
                # vT over full sequence: per m-tile, all heads side by side
                for mt in range(MT):
                    ps = ps1.tile([128, 512], F32, tag="ps1")
                    for cc in range(CG // 2):
                        nc.tensor.matmul(
                            ps,
                            x8[:, 2 * cc:2 * cc + 2, mt * 128:(mt + 1) * 128],
                            wv_sb[:, 2 * cc:2 * cc + 2, :],
                            start=(cc == 0), stop=(cc == CG // 2 - 1),
                            perf_mode=DR)
                    nc.vector.tensor_copy(
                        out=vT[:, mt, :].rearrange("p (h e) -> p h e", e=65)[:, :, 0:64],
                        in_=ps.rearrange("p (h e) -> p h e", e=64))

            # ---------------- Phase 2: attention (bf16, ACT = exp only) ----
            with tc.tile_pool(name="eTp", bufs=4) as eTp, \
                 tc.tile_pool(name="nrm", bufs=2) as nrm, \
                 tc.tile_pool(name="nrm_dram", bufs=2, space="DRAM") as nrm_dram, \
                 tc.tile_pool(name="ps_sc", bufs=2, space="PSUM") as ps_sc, \
                 tc.tile_pool(name="ps_o", bufs=4, space="PSUM") as ps_o:
                for hp in range(NH // 2):
                    hA, hB = 2 * hp, 2 * hp + 1
                    for qc in range(NLOC // 512):
                        oA = ps_o.tile([65, 512], F32, tag="po")
                        oB = ps_o.tile([65, 512], F32, tag="po")
                        for mt in range(MT):
                            sc = ps_sc.tile([128, 1024], F32, tag="sc")
                            nc.tensor.matmul(
                                sc[:, 0:512],
                                k_sb[0:64, hp, mt * 128:(mt + 1) * 128],
                                q_sb[0:64, hp, qc * 512:(qc + 1) * 512],
                                start=True, stop=True, tile_position=(0, 0))
                            nc.tensor.matmul(
                                sc[:, 512:1024],
                                k_sb[64:128, hp, mt * 128:(mt + 1) * 128],
                                q_sb[64:128, hp, qc * 512:(qc + 1) * 512],
                                start=True, stop=True, tile_position=(64, 0))
                            eT = eTp.tile([128, 1024], BF16, tag="eT")
                            nc.scalar.activation(eT, sc, AF.Exp, scale=SC_EXP)
                            nc.tensor.matmul(
                                oA, vT[:, mt, hA * 65:(hA + 1) * 65], eT[:, 0:512],
                                start=(mt == 0), stop=(mt == MT - 1))
                            nc.tensor.matmul(
                                oB, vT[:, mt, hB * 65:(hB + 1) * 65], eT[:, 512:1024],
                                start=(mt == 0), stop=(mt == MT - 1))
                        # normalize by the ones-row sums and place into attnout8
                        r = nrm.tile([1, 1024], F32, tag="r")
                        nc.vector.reciprocal(r[:, 0:512], oA[64:65, :])
                        nc.vector.reciprocal(r[:, 512:1024], oB[64:65, :])
                        r_dram = nrm_dram.tile([1, 1024], F32, tag="rd")
                        nc.sync.dma_start(out=r_dram, in_=r[0:1, :])
                        bc = nrm.tile([64, 1024], F32, tag="bc")
                        rsrc = r_dram[0:1, :]
                        bsrc = bass.AP(tensor=rsrc.tensor, offset=rsrc.offset,
                                       ap=[[0, 64]] + [list(p) for p in rsrc.ap[1:]])
                        nc.sync.dma_start(out=bc, in_=bsrc)
                        nc.vector.tensor_tensor(
                            out=attnout8[0:64, hp, qc * 512:(qc + 1) * 512],
                            in0=oA[0:64, :], in1=bc[:, 0:512], op=ALU.mult)
                        tmpB = nrm.tile([64, 512], FP8, tag="tb")
                        nc.vector.tensor_tensor(
                            out=tmpB, in0=oB[0:64, :], in1=bc[:, 512:1024], op=ALU.mult)
                        nc.sync.dma_start(
                            out=attnout8[64:128, hp, qc * 512:(qc + 1) * 512], in_=tmpB)

        # ---------------- Phase 3-5: proj + BN1, MLP, BN2 (fp8 DR) --------
        with tc.tile_pool(name="mlpw", bufs=1) as mw, \
             tc.tile_pool(name="outp", bufs=2) as outp, \
             tc.tile_pool(name="ps_mm", bufs=4, space="PSUM") as ps_mm:
            wp_sb = mw.tile([128, CG, C], FP8)
            for c in range(CG):
                nc.gpsimd.dma_start(out=wp_sb[:, c, :], in_=wp_d[c * 128:(c + 1) * 128, :])
            w1_sb = mw.tile([128, CG, HID], FP8)
            for c in range(CG):
                nc.gpsimd.dma_start(out=w1_sb[:, c, :], in_=w1_d[c * 128:(c + 1) * 128, :])
            w2_sb = mw.tile([128, HID // 128, C], FP8)
            for c in range(HID // 128):
                nc.gpsimd.dma_start(out=w2_sb[:, c, :], in_=w2_d[c * 128:(c + 1) * 128, :])
            h_sb = mw.tile([128, HID // 128, NLOC], FP8)
            y18 = mw.tile([128, CG, NLOC], FP8)

            # proj + BN1 (+ residual x): y1 = (x + proj/(WS*AS))*bns + bnb
            #   = ps * s1 + bnb + xbn          (s1 = bns/(WS*AS), xbn = x*bns)
            for g in range(CG):
                ps = ps_mm.tile([128, NLOC], F32, tag="mm")
                for cc in range(CG // 2):
                    for qc in range(NLOC // 512):
                        nc.tensor.matmul(
                            ps[:, qc * 512:(qc + 1) * 512],
                            wp_sb[:, 2 * cc:2 * cc + 2, g * 128:(g + 1) * 128],
                            attnout8[:, 2 * cc:2 * cc + 2, qc * 512:(qc + 1) * 512],
                            start=(cc == 0), stop=(cc == CG // 2 - 1),
                            perf_mode=DR)
                t = outp.tile([128, NLOC], F32, tag="t1")
                nc.vector.tensor_scalar(out=t, in0=ps,
                                        scalar1=s1_sb[:, g:g + 1],
                                        scalar2=bnb_sb[:, g:g + 1],
                                        op0=ALU.mult, op1=ALU.add)
                y1 = outp.tile([128, NLOC], F32, tag="y1f")
                nc.vector.tensor_tensor(out=y1, in0=t, in1=xbn[:, g, :], op=ALU.add)
                nc.vector.tensor_copy(out=y18[:, g, :], in_=y1)
                nc.vector.tensor_scalar(out=z1[:, g, :], in0=y1,
                                        scalar1=bns_sb[:, g:g + 1],
                                        scalar2=None, op0=ALU.mult)
            # fc1 + relu (relu on DVE: max(x, 0))
            for go in range(HID // 128):
                ps = ps_mm.tile([128, NLOC], F32, tag="mm")
                for cc in range(CG // 2):
                    for qc in range(NLOC // 512):
                        nc.tensor.matmul(
                            ps[:, qc * 512:(qc + 1) * 512],
                            w1_sb[:, 2 * cc:2 * cc + 2, go * 128:(go + 1) * 128],
                            y18[:, 2 * cc:2 * cc + 2, qc * 512:(qc + 1) * 512],
                            start=(cc == 0), stop=(cc == CG // 2 - 1),
                            perf_mode=DR)
                nc.vector.tensor_scalar(out=h_sb[:, go, :], in0=ps,
                                        scalar1=0.0, scalar2=None, op0=ALU.max)
            # fc2 + BN2 (+ residual y1): y = (y1 + fc2/(WS*WS))*bns + bnb
            #   = ps * s2 + bnb + z1           (s2 = bns/(WS*WS), z1 = y1*bns)
            for g in range(CG):
                ps = ps_mm.tile([128, NLOC], F32, tag="mm")
                for hc in range(HID // 256):
                    for qc in range(NLOC // 512):
                        nc.tensor.matmul(
                            ps[:, qc * 512:(qc + 1) * 512],
                            w2_sb[:, 2 * hc:2 * hc + 2, g * 128:(g + 1) * 128],
                            h_sb[:, 2 * hc:2 * hc + 2, qc * 512:(qc + 1) * 512],
                            start=(hc == 0), stop=(hc == HID // 256 - 1),
                            perf_mode=DR)
                t2 = outp.tile([128, NLOC], F32, tag="t2")
                nc.vector.tensor_scalar(out=t2, in0=ps,
                                        scalar1=s2_sb[:, g:g + 1],
                                        scalar2=bnb_sb[:, g:g + 1],
                                        op0=ALU.mult, op1=ALU.add)
                ob = outp.tile([128, NLOC], F32, tag="ob")
                nc.vector.tensor_tensor(out=ob, in0=t2, in1=z1[:, g, :], op=ALU.add)
                nc.sync.dma_start(out=y_d[g * 128:(g + 1) * 128, :], in_=ob)

    with tile.TileContext(nc) as tc, ExitStack() as ctx:
        pers = ctx.enter_context(tc.tile_pool(name="pers", bufs=1))

        xbn = pers.tile([128, CG, NLOC], F32R)
        for g in range(CG):
            nc.sync.dma_start(out=xbn[:, g, :], in_=xbn_d[g * 128:(g + 1) * 128, :])
        bns_sb = pers.tile([128, CG], F32)
        bnb_sb = pers.tile([128, CG], F32)
        s1_sb = pers.tile([128, CG], F32)
        s2_sb = pers.tile([128, CG], F32)
        for g in range(CG):
            nc.sync.dma_start(out=bns_sb[:, g:g + 1], in_=bns_d[g * 128:(g + 1) * 128, :])
            nc.sync.dma_start(out=bnb_sb[:, g:g + 1], in_=bnb_d[g * 128:(g + 1) * 128, :])
            nc.sync.dma_start(out=s1_sb[:, g:g + 1], in_=s1_d[g * 128:(g + 1) * 128, :])
            nc.sync.dma_start(out=s2_sb[:, g:g + 1], in_=s2_d[g * 128:(g + 1) * 128, :])
        attnout8 = pers.tile([128, CG, NLOC], FP8)
        z1 = pers.tile([128, CG, NLOC], F32R)

        for _rep in range(repeat):
            emit_body(tc, (xbn, bns_sb, bnb_sb, s1_sb, s2_sb, attnout8, z1))

    nc.compile()
    return nc


def _host_prep(x, w_qkv, w_proj, w_fc1, w_fc2, gamma, beta, running_mean, running_var):
    import ml_dtypes
    FP8NP = ml_dtypes.float8_e4m3fn
    x = np.asarray(x, np.float32)
    w_qkv = np.asarray(w_qkv, np.float32)
    bns = (np.asarray(gamma, np.float32)
           / np.sqrt(np.asarray(running_var, np.float32) + BN_EPS))
    bnb = np.asarray(beta, np.float32) - np.asarray(running_mean, np.float32) * bns
    s1 = bns / (WS * AS)
    s2 = bns / (WS * WS)
    wqT = (np.ascontiguousarray(w_qkv[0:C].T) * WS).astype(FP8NP)
    wkT = (np.ascontiguousarray(w_qkv[C:2 * C].T) * WS).astype(FP8NP)
    wvT = (np.ascontiguousarray(w_qkv[2 * C:3 * C].T) * WS).astype(FP8NP)
    wpT = (np.ascontiguousarray(np.asarray(w_proj, np.float32).T) * WS).astype(FP8NP)
    w1T = (np.ascontiguousarray(np.asarray(w_fc1, np.float32).T) * WS).astype(FP8NP)
    w2T = (np.ascontiguousarray(np.asarray(w_fc2, np.float32).T) * WS).astype(FP8NP)
    ones = np.full((128, MT * NH), ONESV, ml_dtypes.bfloat16)
    common = dict(wqT=wqT, wkT=wkT, wvT=wvT, wpT=wpT, w1T=w1T, w2T=w2T,
                  bns=bns.reshape(C, 1).astype(np.float32),
                  bnb=bnb.reshape(C, 1).astype(np.float32),
                  s1=s1.reshape(C, 1).astype(np.float32),
                  s2=s2.reshape(C, 1).astype(np.float32),
                  ones=ones)
    in_maps = []
    for core in range(8):
        b, s = core // 2, core % 2
        xr = np.ascontiguousarray(np.roll(x[b], -s * NLOC, axis=1))
        x8 = xr.astype(FP8NP)
        xbn = np.ascontiguousarray(xr[:, 0:NLOC]) * bns[:, None]
        in_maps.append(dict(x8=x8, xbn=xbn.astype(np.float32), **common))
    return x, in_maps


def kernel(x, w_qkv, w_proj, w_fc1, w_fc2, gamma, beta,
           running_mean, running_var, **_ignored):
    from concourse.bass_utils import run_bass_kernel_spmd
    if 'nc' not in _CACHE:
        _CACHE['nc'] = _build_nc()
    nc = _CACHE['nc']
    x, in_maps = _host_prep(x, w_qkv, w_proj, w_fc1, w_fc2, gamma, beta,
                            running_mean, running_var)
    res = run_bass_kernel_spmd(nc, in_maps, core_ids=list(range(8)))
    y = np.empty((B, C, N), np.float32)
    for core in range(8):
        b, s = core // 2, core % 2
        y[b][:, s * NLOC:(s + 1) * NLOC] = res.results[core]["y"]
    return y
